# revision 22
# baseline (speedup 1.0000x reference)
"""Trainium2 Bass kernel for nn_AttLayer (sliding-block attention encoder layer).

Sharding: 8 cores = 4 batches x 2 sequence halves (4096 frames each).
Each core gets its x1 slice with a 256-frame halo on both sides (zero-padded at
sequence edges), computes q/k/v projections, 8 blocks of windowed attention
(block 512, window 1024), relu + output projection locally. No collectives.

Device layout choices:
  - all matmul operands in BF16 (x, weights host-cast; q/k/pt/v/rl via
    PSUM-evacuation output dtype). PSUM accumulation stays fp32.
  - q, k stored [c=256(2 ptiles), Lext=4608] in SBUF.
  - v stored TRANSPOSED [Lext(36 ptiles), c3=256]  (computed directly as
    x^T @ Wv^T so no on-chip transpose is ever needed).
  - energy computed transposed: eT[k, q] = sum_c k[c,k] q[c,q]  -> the softmax
    log-mask bias lands on the partition dim, a perfect fit for the ACT
    engine's per-partition bias operand:  P = Exp(eT/16 + bias).
  - no max-subtraction in softmax (energies are O(10), exp is safe in fp32).
  - row sums over the 8 exp tiles via a 3-level pairwise bf16 tree on DVE,
    then a [128,128] ones matmul that reduces partitions AND replicates the
    sum to all 128 partitions (so no gpsimd partition_broadcast is needed);
    reciprocal runs directly on that [128, 512] PSUM.
  - relu & normalization fused in one DVE op via the identity
    relu(o/s) = relu(o)*(1/s):   rl = (o max 0) * rb   (scalar_tensor_tensor).
    When bv != 0 a general 2-op variant is built instead (lazy, cached).
  - out-projection PSUM is evacuated by plain copies spread over the scalar /
    vector / gpsimd engines; bo is added on the host after gathering.
  - qk and av matmuls are software-interleaved (qk 4 ahead) so the Exp
    evacuations keep pace with the energy-PSUM ring.
  - keep-warm dummy matmuls in the pipeline drain so the last block's output
    projection doesn't run at the PE's cold pstate.
"""

import numpy as np

# problem constants (self-contained; must match the harness reference)
B, CIN, L = 4, 512, 8192
C, VD = 256, 512
BL, HALF = 512, 256
NCORES = 8
LCH = L // 2            # 4096 frames per core
LEXT = LCH + 2 * HALF   # 4608 with halo
NBLK = LCH // BL        # 8 local blocks
WS = BL + 2 * HALF      # 1024 window
NKT = WS // 128         # 8 k-tiles per window
NCH = LEXT // BL        # 9 x chunks
NVT = LEXT // 128       # 36 v^T partition tiles

_NC_CACHE = {}


def _build_nc(bv_zero, ab_paired=True):
    import concourse.bacc as bacc
    import concourse.mybir as mybir
    import concourse.tile as tile
    from contextlib import ExitStack

    f32 = mybir.dt.float32
    bf16 = mybir.dt.bfloat16
    rdt = bf16
    AF = mybir.ActivationFunctionType
    ALU = mybir.AluOpType

    nc = bacc.Bacc("TRN2", target_bir_lowering=False, debug=False,
                   num_devices=NCORES)

    x_d = nc.dram_tensor("x", [CIN, LEXT], rdt, kind="ExternalInput").ap()
    wq_d = nc.dram_tensor("wq_t", [CIN, C], rdt, kind="ExternalInput").ap()
    wk_d = nc.dram_tensor("wk_t", [CIN, C], rdt, kind="ExternalInput").ap()
    wv_d = nc.dram_tensor("wv_t", [CIN, C], rdt, kind="ExternalInput").ap()
    wo_d = nc.dram_tensor("wo_t", [C, VD], rdt, kind="ExternalInput").ap()
    bq_d = nc.dram_tensor("bq", [C, 1], f32, kind="ExternalInput").ap()
    bk_d = nc.dram_tensor("bk", [C, 1], f32, kind="ExternalInput").ap()
    bv_d = nc.dram_tensor("bv", [C, 1], f32, kind="ExternalInput").ap()
    ab_d = nc.dram_tensor("abias", [128, NBLK * NKT], f32,
                          kind="ExternalInput").ap()
    out_d = nc.dram_tensor("out", [VD, LCH], rdt, kind="ExternalOutput").ap()

    x_r = x_d.rearrange("(r p) l -> p r l", p=128)      # [128, 4, 4608]
    wq_r = wq_d.rearrange("(r p) c -> p r c", p=128)    # [128, 4, 256]
    wk_r = wk_d.rearrange("(r p) c -> p r c", p=128)
    wv_r = wv_d.rearrange("(r p) c -> p r c", p=128)
    wo_r = wo_d.rearrange("(m p) v -> p m v", p=128)    # [128, 2, 512]
    bq_r = bq_d.rearrange("(m p) o -> p m o", p=128)    # [128, 2, 1]
    bk_r = bk_d.rearrange("(m p) o -> p m o", p=128)
    bv_r = bv_d.rearrange("(m p) o -> p m o", p=128)
    out_r = out_d.rearrange("(v p) l -> p v l", p=128)  # [128, 4, 4096]

    with tile.TileContext(nc) as tc:
        with ExitStack() as ctx:
            ctx.enter_context(nc.allow_low_precision(
                reason="bf16 matmul pipeline; fp32 PSUM accumulation"))
            sbc = ctx.enter_context(tc.tile_pool(name="sbc", bufs=1))  # constants
            sbp = ctx.enter_context(tc.tile_pool(name="sbp", bufs=1))  # persistent
            sbs = ctx.enter_context(tc.tile_pool(name="sbs", bufs=1))  # streaming
            ps = ctx.enter_context(tc.tile_pool(name="ps", bufs=1, space="PSUM"))

            dma = nc.sync.dma_start

            wq = sbc.tile([128, 4, C], rdt, tag="wq", name="wq")
            wk = sbc.tile([128, 4, C], rdt, tag="wk", name="wk")
            wv = sbc.tile([128, 4, C], rdt, tag="wv", name="wv")
            xt0 = sbs.tile([128, 4, BL], rdt, tag="x", bufs=3, name="xt0")
            xt1 = sbs.tile([128, 4, BL], rdt, tag="x", bufs=3, name="xt1")
            bq = sbc.tile([128, 2, 1], f32, tag="bq", name="bq")
            bk = sbc.tile([128, 2, 1], f32, tag="bk", name="bk")
            # chunk-0 q only consumes cols [HALF:BL]; land that piece first so
            # the first projection matmul can start ~1.4us earlier.
            dma(out=xt0[:, :, HALF:BL], in_=x_r[:, :, HALF:BL])
            dma(out=wq[:], in_=wq_r)
            dma(out=bq[:], in_=bq_r)
            dma(out=xt0[:, :, 0:HALF], in_=x_r[:, :, 0:HALF])
            dma(out=wk[:], in_=wk_r)
            dma(out=xt1[:], in_=x_r[:, :, BL:2 * BL])
            dma(out=bk[:], in_=bk_r)
            xt2 = sbs.tile([128, 4, BL], rdt, tag="x", bufs=3, name="xt2")
            dma(out=wv[:], in_=wv_r)
            dma(out=xt2[:], in_=x_r[:, :, 2 * BL:3 * BL])

            # PE warmup: dependency-free bf16 matmuls during the DMA head
            # so the HAM clock-gate reaches 8/8 before real work arrives.
            wrm = sbc.tile([128, BL], bf16, tag="wrm", name="wrm")
            nc.vector.memset(wrm[:], 0.0)
            wps = ps.tile([128, BL], f32, tag="pp", bufs=2, name="wps")
            for i in range(10):
                nc.tensor.matmul(wps[:], wrm[:, 0:128], wrm[:], start=True,
                                 stop=True)

            def emit_late_consts():
                wo = sbc.tile([128, 2, VD], rdt, tag="wo", name="wo")
                dma(out=wo[:], in_=wo_r)
                bv = sbc.tile([128, 2, 1], f32, tag="bv", name="bv")
                dma(out=bv[:], in_=bv_r)
                ab = sbc.tile([128, NBLK * NKT], f32, tag="ab", name="ab")
                dma(out=ab[:], in_=ab_d)
                ones_f = sbc.tile([128, 128], f32, tag="ones_f", name="ones_f")
                nc.vector.memset(ones_f[:], 1.0)
                ones_m = sbc.tile([128, 128], rdt, tag="ones_m", name="ones_m")
                nc.vector.tensor_copy(ones_m[:], ones_f[:])
                return wo, bv, ab, ones_m

            q_sb = [sbp.tile([128, LEXT], rdt, tag=f"qsb{i}", name=f"qsb{i}")
                    for i in range(2)]
            k_sb = [sbp.tile([128, LEXT], rdt, tag=f"ksb{i}", name=f"ksb{i}")
                    for i in range(2)]
            # v^T tiles, paired: [128, 2, C] so the PSUM evacuation is one op
            vts = [sbp.tile([128, 2, C], rdt, tag=f"vt{i}", name=f"vt{i}")
                   for i in range(NVT // 2)]

            def mm(out_ap, lhsT, rhs, start, stop):
                nc.tensor.matmul(out_ap, lhsT, rhs, start=start, stop=stop)

            # paired-exp builds use 2-bank "ep" slots (2 bufs); the fallback
            # uses 1-bank "e" slots (4 bufs). One tag per build keeps the
            # PSUM budget at 8 banks: pp(2) + energy(4) + o0 + o1.
            ptag = "ep" if ab_paired else "e"
            pbufs = 2 if ab_paired else 4

            # ---------------- projections (streamed over 9 x-chunks) --------
            def emit_chunk(c):
                if c == 0:
                    xt = xt0
                elif c == 1:
                    xt = xt1
                elif c == 2:
                    xt = xt2
                else:
                    xt = sbs.tile([128, 4, BL], rdt, tag="x", bufs=3,
                                  name=f"xt{c}")
                    dma(out=xt[:], in_=x_r[:, :, c * BL:(c + 1) * BL])
                # q is only needed on extended cols [HALF, LEXT-HALF)
                qlo = max(c * BL, HALF) - c * BL
                qhi = min((c + 1) * BL, LEXT - HALF) - c * BL
                for o in range(2):
                    pq = ps.tile([128, BL], f32, tag="pp", bufs=2,
                                 name=f"pq{c}_{o}")
                    for r in range(4):
                        mm(pq[:, 0:qhi - qlo],
                           wq[:, r, o * 128:(o + 1) * 128],
                           xt[:, r, qlo:qhi], r == 0, r == 3)
                    # split the two q evacuations across scalar/vector so
                    # the pp-ring drains in parallel
                    if o == 0:
                        nc.scalar.activation(
                            q_sb[o][:, c * BL + qlo:c * BL + qhi],
                            pq[:, 0:qhi - qlo], AF.Identity,
                            bias=bq[:, o, :], scale=1.0)
                    else:
                        nc.vector.tensor_scalar_add(
                            q_sb[o][:, c * BL + qlo:c * BL + qhi],
                            pq[:, 0:qhi - qlo], bq[:, o, :])
                for o in range(2):
                    pk = ps.tile([128, BL], f32, tag="pp", bufs=2,
                                 name=f"pk{c}_{o}")
                    for r in range(4):
                        mm(pk[:], wk[:, r, o * 128:(o + 1) * 128], xt[:, r, :],
                           r == 0, r == 3)
                    nc.vector.tensor_scalar_add(
                        k_sb[o][:, c * BL:(c + 1) * BL], pk[:], bk[:, o, :])
                for lp in range(2):
                    pv = ps.tile([128, 2, C], f32, tag=ptag, bufs=pbufs,
                                 name=f"pv{c}_{lp}")
                    for j in range(2):
                        lt = lp * 2 + j
                        for r in range(4):
                            mm(pv[:, j, :], xt[:, r, lt * 128:(lt + 1) * 128],
                               wv[:, r, :], r == 0, r == 3)
                    nc.vector.tensor_copy(vts[c * 2 + lp][:], pv[:])

            emit_chunk(0)
            wo, bv, ab, ones_m = emit_late_consts()
            for c in range(1, NCH):
                emit_chunk(c)

            # ---------------- attention (software-pipelined blocks) ---------
            OPS, SPS, RBS, ORL = {}, {}, {}, {}

            def emit_attn(b):
                pts = sbs.tile([128, NKT, BL], rdt, tag="pt", bufs=2,
                               name=f"pt{b}")
                o0 = ps.tile([128, BL], f32, tag="o0", bufs=1, name=f"o0_{b}")
                o1 = ps.tile([128, BL], f32, tag="o1", bufs=1, name=f"o1_{b}")

                def qk(kt):
                    pe = ps.tile([128, BL], f32, tag="e", bufs=4,
                                 name=f"e{b}_{kt}")
                    for ct in range(2):
                        mm(pe[:],
                           k_sb[ct][:, b * BL + kt * 128:
                                    b * BL + (kt + 1) * 128],
                           q_sb[ct][:, HALF + b * BL:HALF + (b + 1) * BL],
                           ct == 0, ct == 1)
                    nc.scalar.activation(
                        pts[:, kt, :], pe[:], AF.Exp,
                        bias=ab[:, b * NKT + kt:b * NKT + kt + 1],
                        scale=1.0 / 16.0)

                def qk_pair(kp):
                    # 2-bank energy tile; one Exp ACT covers both k-tiles
                    # (the log-mask bias is per-partition and pairwise-equal
                    # for every mask setup_inputs produces - host-verified)
                    pe = ps.tile([128, 2, BL], f32, tag="ep", bufs=2,
                                 name=f"ep{b}_{kp}")
                    for j in range(2):
                        kt = 2 * kp + j
                        for ct in range(2):
                            mm(pe[:, j, :],
                               k_sb[ct][:, b * BL + kt * 128:
                                        b * BL + (kt + 1) * 128],
                               q_sb[ct][:, HALF + b * BL:HALF + (b + 1) * BL],
                               ct == 0, ct == 1)
                    nc.scalar.activation(
                        pts[:, 2 * kp:2 * kp + 2, :], pe[:], AF.Exp,
                        bias=ab[:, b * NKT + 2 * kp:b * NKT + 2 * kp + 1],
                        scale=1.0 / 16.0)

                def av(kt):
                    vtt = vts[(b * 4 + kt) // 2]
                    j = (b * 4 + kt) % 2
                    pt = pts[:, kt, :]
                    mm(o0[:], vtt[:, j, 0:128], pt, kt == 0, kt == NKT - 1)
                    mm(o1[:], vtt[:, j, 128:256], pt, kt == 0, kt == NKT - 1)

                if ab_paired:
                    qk_pair(0)
                    qk_pair(1)
                    av(0)
                    qk_pair(2)
                    av(1)
                    av(2)
                    qk_pair(3)
                    for kt in range(3, NKT):
                        av(kt)
                else:
                    for kt in range(4):
                        qk(kt)
                    for kt in range(NKT):
                        av(kt)
                        if kt + 4 < NKT:
                            qk(kt + 4)

                # pairwise bf16 tree for the row sums
                t4 = sbs.tile([128, 4, BL], rdt, tag="t4", bufs=2,
                              name=f"t4_{b}")
                nc.vector.tensor_add(t4[:], pts[:, 0:NKT:2, :],
                                     pts[:, 1:NKT:2, :])
                t2 = sbs.tile([128, 2, BL], rdt, tag="t2", bufs=2,
                              name=f"t2_{b}")
                nc.vector.tensor_add(t2[:], t4[:, 0:4:2, :], t4[:, 1:4:2, :])
                sar = sbs.tile([128, BL], rdt, tag="sar", bufs=2,
                               name=f"sar{b}")
                nc.vector.tensor_add(sar[:], t2[:, 0, :], t2[:, 1, :])
                OPS[b] = (o0, o1)
                SPS[b] = sar

            def emit_finA(b):
                # [128,128] ones lhsT: reduces partitions AND replicates the
                # row-sum to all 128 partitions (no gpsimd broadcast needed).
                # Emitted after outproj(b-1) so the tree has ~1.7us of slack
                # before the PE reaches this matmul.
                sp = ps.tile([128, BL], f32, tag="pp", bufs=2, name=f"s{b}")
                mm(sp[:], ones_m[:], SPS[b][:], True, True)
                rb = sbs.tile([128, BL], f32, tag="rbs", bufs=2, name=f"rb{b}")
                nc.vector.reciprocal_approx_fast(rb[:], sp[:])
                RBS[b] = rb

            def emit_normrelu(b):
                orl = []
                for m in range(2):
                    rl = sbs.tile([128, BL], rdt, tag=f"rl{m}", bufs=2,
                                  name=f"rl{b}_{m}")
                    if bv_zero:
                        # relu(o/s + 0) == relu(o) * (1/s)   (s > 0)
                        nc.vector.scalar_tensor_tensor(
                            rl[:], OPS[b][m][:], 0.0, RBS[b][:],
                            ALU.max, ALU.mult)
                    else:
                        on = sbs.tile([128, BL], f32, tag=f"on{m}", bufs=2,
                                      name=f"on{b}_{m}")
                        nc.vector.tensor_mul(on[:], OPS[b][m][:], RBS[b][:])
                        nc.vector.tensor_scalar(
                            rl[:], on[:], bv[:, m, :], 0.0, ALU.add, ALU.max)
                    orl.append(rl)
                ORL[b] = orl

            def emit_outproj(b):
                ob = sbs.tile([128, 4, BL], rdt, tag="ob", bufs=2,
                              name=f"ob{b}")
                for v in range(4):
                    po = ps.tile([128, BL], f32, tag="pp", bufs=2,
                                 name=f"po{b}_{v}")
                    for m in range(2):
                        mm(po[:], wo[:, m, v * 128:(v + 1) * 128],
                           ORL[b][m][:], m == 0, m == 1)
                    # bo is added on the host; spread the evacuation copies
                    # across engines (gpsimd cannot read PSUM), and DMA each
                    # v-slice as it lands so the tail transfer is small
                    if v in (1, 3):
                        nc.scalar.copy(ob[:, v, :], po[:])
                    else:
                        nc.vector.tensor_copy(ob[:, v, :], po[:])
                    dma(out=out_r[:, v:v + 1, b * BL:(b + 1) * BL],
                        in_=ob[:, v:v + 1, :])

            for step in range(NBLK + 1):
                if step == NBLK:
                    # pipeline drain: keep the PE pstate hot with
                    # dependency-free dummies so the last outproj runs at
                    # full clock instead of the cold 2-3x-slower rate.
                    dps = ps.tile([128, BL], f32, tag=ptag, bufs=pbufs,
                                  name="dummy")
                    for i in range(20):
                        nc.tensor.matmul(dps[:, 0:256], wrm[:, 0:128],
                                         wrm[:, 0:256], start=True, stop=True)
                    for i in range(20):
                        nc.tensor.matmul(dps[:, 0:64], wrm[:, 0:128],
                                         wrm[:, 0:64], start=True, stop=True)
                if 1 <= step <= NBLK:
                    emit_normrelu(step - 1)
                if step < NBLK:
                    emit_attn(step)
                if 1 <= step <= NBLK:
                    emit_outproj(step - 1)
                if step < NBLK:
                    emit_finA(step)

    nc.compile()
    return nc


def get_nc(bv_zero=True, ab_paired=True):
    key = ("bf16", bv_zero, ab_paired)
    if key not in _NC_CACHE:
        _NC_CACHE[key] = _build_nc(bv_zero, ab_paired)
    return _NC_CACHE[key]


def make_core_inputs(inputs):
    """Split full inputs into 8 per-core input maps."""
    import ml_dtypes
    bf16 = ml_dtypes.bfloat16

    x1 = np.asarray(inputs["x1"], dtype=np.float32)
    mask = np.asarray(inputs["mask"], dtype=np.float32)
    wq_t = np.ascontiguousarray(
        np.asarray(inputs["Wq"], np.float32).T.astype(bf16))
    wk_t = np.ascontiguousarray(
        np.asarray(inputs["Wk"], np.float32).T.astype(bf16))
    wv_t = np.ascontiguousarray(
        np.asarray(inputs["Wv"], np.float32).T.astype(bf16))
    wo_t = np.ascontiguousarray(
        np.asarray(inputs["Wo"], np.float32).T.astype(bf16))
    bq = np.asarray(inputs["bq"], np.float32).reshape(C, 1)
    bk = np.asarray(inputs["bk"], np.float32).reshape(C, 1)
    bv = np.asarray(inputs["bv"], np.float32).reshape(C, 1)

    # padded log-mask (the reference pads mask with zeros, then adds
    # log(mask + 1e-6) to the energies)
    mp = np.pad(mask[:, 0, :], ((0, 0), (HALF, HALF)))
    lb = np.log(mp + np.float32(1e-6)).astype(np.float32)  # [B, L + 2*HALF]

    in_maps = []
    for core in range(NCORES):
        b, h = divmod(core, 2)
        s = h * LCH
        xe = np.zeros((CIN, LEXT), bf16)
        lo, hi = s - HALF, s + LCH + HALF
        slo, shi = max(lo, 0), min(hi, L)
        xe[:, slo - lo:slo - lo + (shi - slo)] = \
            x1[b, :, slo:shi].astype(bf16)
        ab = np.empty((128, NBLK * NKT), np.float32)
        for blk in range(NBLK):
            w = lb[b, s + blk * BL:s + blk * BL + WS]
            ab[:, blk * NKT:(blk + 1) * NKT] = w.reshape(NKT, 128).T
        in_maps.append({
            "x": xe, "wq_t": wq_t, "wk_t": wk_t, "wv_t": wv_t, "wo_t": wo_t,
            "bq": bq, "bk": bk, "bv": bv, "abias": ab,
        })
    return in_maps


def assemble_output(results, bo):
    out = np.empty((B, VD, L), np.float32)
    bo_col = np.asarray(bo, np.float32).reshape(VD, 1)
    for core in range(NCORES):
        b, h = divmod(core, 2)
        out[b, :, h * LCH:(h + 1) * LCH] = \
            results[core]["out"].astype(np.float32) + bo_col
    return out


LAST_RESULT = None


def kernel(**inputs):
    global LAST_RESULT
    from concourse.bass_utils import run_bass_kernel_spmd

    bv_zero = bool(np.all(np.asarray(inputs["bv"]) == 0.0))
    in_maps = make_core_inputs(inputs)
    ab0 = in_maps[0]["abias"]
    ab_paired = all(
        np.array_equal(m["abias"][:, 0::2], m["abias"][:, 1::2])
        for m in in_maps)
    nc = get_nc(bv_zero, ab_paired)
    res = run_bass_kernel_spmd(nc, in_maps, list(range(NCORES)))
    LAST_RESULT = res
    return assemble_output(res.results, inputs["bo"])


# revision 23
# speedup vs baseline: 1.0181x; 1.0181x over previous
"""Trainium2 Bass kernel for nn_AttLayer (sliding-block attention encoder layer).

Sharding: 8 cores = 4 batches x 2 sequence halves (4096 frames each).
Each core gets its x1 slice with a 256-frame halo on both sides (zero-padded at
sequence edges), computes q/k/v projections, 8 blocks of windowed attention
(block 512, window 1024), relu + output projection locally. No collectives.

Device layout choices:
  - all matmul operands in BF16 (x, weights host-cast; q/k/pt/v/rl via
    PSUM-evacuation output dtype). PSUM accumulation stays fp32.
  - q, k stored [c=256(2 ptiles), Lext=4608] in SBUF.
  - v stored TRANSPOSED [Lext(36 ptiles), c3=256]  (computed directly as
    x^T @ Wv^T so no on-chip transpose is ever needed).
  - energy computed transposed: eT[k, q] = sum_c k[c,k] q[c,q]  -> the softmax
    log-mask bias lands on the partition dim, a perfect fit for the ACT
    engine's per-partition bias operand:  P = Exp(eT/16 + bias).
  - no max-subtraction in softmax (energies are O(10), exp is safe in fp32).
  - row sums over the 8 exp tiles via a 3-level pairwise bf16 tree on DVE,
    then a [128,128] ones matmul that reduces partitions AND replicates the
    sum to all 128 partitions (so no gpsimd partition_broadcast is needed);
    reciprocal runs directly on that [128, 512] PSUM.
  - relu & normalization fused in one DVE op via the identity
    relu(o/s) = relu(o)*(1/s):   rl = (o max 0) * rb   (scalar_tensor_tensor).
    When bv != 0 a general 2-op variant is built instead (lazy, cached).
  - out-projection PSUM is evacuated by plain copies spread over the scalar /
    vector / gpsimd engines; bo is added on the host after gathering.
  - qk and av matmuls are software-interleaved (qk 4 ahead) so the Exp
    evacuations keep pace with the energy-PSUM ring.
  - keep-warm dummy matmuls in the pipeline drain so the last block's output
    projection doesn't run at the PE's cold pstate.
"""

import numpy as np

# problem constants (self-contained; must match the harness reference)
B, CIN, L = 4, 512, 8192
C, VD = 256, 512
BL, HALF = 512, 256
NCORES = 8
LCH = L // 2            # 4096 frames per core
LEXT = LCH + 2 * HALF   # 4608 with halo
NBLK = LCH // BL        # 8 local blocks
WS = BL + 2 * HALF      # 1024 window
NKT = WS // 128         # 8 k-tiles per window
NCH = LEXT // BL        # 9 x chunks
NVT = LEXT // 128       # 36 v^T partition tiles

_NC_CACHE = {}


def _build_nc(bv_zero, ab_paired=True):
    import concourse.bacc as bacc
    import concourse.mybir as mybir
    import concourse.tile as tile
    from contextlib import ExitStack

    f32 = mybir.dt.float32
    bf16 = mybir.dt.bfloat16
    rdt = bf16
    AF = mybir.ActivationFunctionType
    ALU = mybir.AluOpType

    nc = bacc.Bacc("TRN2", target_bir_lowering=False, debug=False,
                   num_devices=NCORES)

    x_d = nc.dram_tensor("x", [CIN, LEXT], rdt, kind="ExternalInput").ap()
    wq_d = nc.dram_tensor("wq_t", [CIN, C], rdt, kind="ExternalInput").ap()
    wk_d = nc.dram_tensor("wk_t", [CIN, C], rdt, kind="ExternalInput").ap()
    wv_d = nc.dram_tensor("wv_t", [CIN, C], rdt, kind="ExternalInput").ap()
    wo_d = nc.dram_tensor("wo_t", [C, VD], rdt, kind="ExternalInput").ap()
    bq_d = nc.dram_tensor("bq", [C, 1], f32, kind="ExternalInput").ap()
    bk_d = nc.dram_tensor("bk", [C, 1], f32, kind="ExternalInput").ap()
    bv_d = nc.dram_tensor("bv", [C, 1], f32, kind="ExternalInput").ap()
    ab_d = nc.dram_tensor("abias", [128, NBLK * NKT], f32,
                          kind="ExternalInput").ap()
    out_d = nc.dram_tensor("out", [VD, LCH], rdt, kind="ExternalOutput").ap()

    x_r = x_d.rearrange("(r p) l -> p r l", p=128)      # [128, 4, 4608]
    wq_r = wq_d.rearrange("(r p) c -> p r c", p=128)    # [128, 4, 256]
    wk_r = wk_d.rearrange("(r p) c -> p r c", p=128)
    wv_r = wv_d.rearrange("(r p) c -> p r c", p=128)
    wo_r = wo_d.rearrange("(m p) v -> p m v", p=128)    # [128, 2, 512]
    bq_r = bq_d.rearrange("(m p) o -> p m o", p=128)    # [128, 2, 1]
    bk_r = bk_d.rearrange("(m p) o -> p m o", p=128)
    bv_r = bv_d.rearrange("(m p) o -> p m o", p=128)
    out_r = out_d.rearrange("(v p) l -> p v l", p=128)  # [128, 4, 4096]

    with tile.TileContext(nc) as tc:
        with ExitStack() as ctx:
            ctx.enter_context(nc.allow_low_precision(
                reason="bf16 matmul pipeline; fp32 PSUM accumulation"))
            sbc = ctx.enter_context(tc.tile_pool(name="sbc", bufs=1))  # constants
            sbp = ctx.enter_context(tc.tile_pool(name="sbp", bufs=1))  # persistent
            sbs = ctx.enter_context(tc.tile_pool(name="sbs", bufs=1))  # streaming
            ps = ctx.enter_context(tc.tile_pool(name="ps", bufs=1, space="PSUM"))

            dma = nc.sync.dma_start

            wq = sbc.tile([128, 4, C], rdt, tag="wq", name="wq")
            wk = sbc.tile([128, 4, C], rdt, tag="wk", name="wk")
            wv = sbc.tile([128, 4, C], rdt, tag="wv", name="wv")
            xt0 = sbs.tile([128, 4, BL], rdt, tag="x", bufs=3, name="xt0")
            xt1 = sbs.tile([128, 4, BL], rdt, tag="x", bufs=3, name="xt1")
            bq = sbc.tile([128, 2, 1], f32, tag="bq", name="bq")
            bk = sbc.tile([128, 2, 1], f32, tag="bk", name="bk")
            # chunk-0 q only consumes cols [HALF:BL]; land that piece first so
            # the first projection matmul can start ~1.4us earlier.
            dma(out=xt0[:, :, HALF:BL], in_=x_r[:, :, HALF:BL])
            dma(out=wq[:], in_=wq_r)
            dma(out=bq[:], in_=bq_r)
            dma(out=xt0[:, :, 0:HALF], in_=x_r[:, :, 0:HALF])
            dma(out=wk[:], in_=wk_r)
            dma(out=xt1[:], in_=x_r[:, :, BL:2 * BL])
            dma(out=bk[:], in_=bk_r)
            xt2 = sbs.tile([128, 4, BL], rdt, tag="x", bufs=3, name="xt2")
            dma(out=wv[:], in_=wv_r)
            dma(out=xt2[:], in_=x_r[:, :, 2 * BL:3 * BL])

            # PE warmup: dependency-free bf16 matmuls during the DMA head
            # so the HAM clock-gate reaches 8/8 before real work arrives.
            wrm = sbc.tile([128, BL], bf16, tag="wrm", name="wrm")
            nc.vector.memset(wrm[:], 0.0)
            wps = ps.tile([128, BL], f32, tag="pp", bufs=2, name="wps")
            for i in range(10):
                nc.tensor.matmul(wps[:], wrm[:, 0:128], wrm[:], start=True,
                                 stop=True)

            def emit_late_consts():
                wo = sbc.tile([128, 2, VD], rdt, tag="wo", name="wo")
                dma(out=wo[:], in_=wo_r)
                bv = sbc.tile([128, 2, 1], f32, tag="bv", name="bv")
                dma(out=bv[:], in_=bv_r)
                ab = sbc.tile([128, NBLK * NKT], f32, tag="ab", name="ab")
                dma(out=ab[:], in_=ab_d)
                ones_f = sbc.tile([128, 128], f32, tag="ones_f", name="ones_f")
                nc.vector.memset(ones_f[:], 1.0)
                ones_m = sbc.tile([128, 128], rdt, tag="ones_m", name="ones_m")
                nc.vector.tensor_copy(ones_m[:], ones_f[:])
                return wo, bv, ab, ones_m

            q_sb = [sbp.tile([128, LEXT], rdt, tag=f"qsb{i}", name=f"qsb{i}")
                    for i in range(2)]
            k_sb = [sbp.tile([128, LEXT], rdt, tag=f"ksb{i}", name=f"ksb{i}")
                    for i in range(2)]
            # v^T tiles, paired: [128, 2, C] so the PSUM evacuation is one op
            vts = [sbp.tile([128, 2, C], rdt, tag=f"vt{i}", name=f"vt{i}")
                   for i in range(NVT // 2)]

            def mm(out_ap, lhsT, rhs, start, stop):
                nc.tensor.matmul(out_ap, lhsT, rhs, start=start, stop=stop)

            # paired-exp builds use 2-bank "ep" slots (2 bufs); the fallback
            # uses 1-bank "e" slots (4 bufs). One tag per build keeps the
            # PSUM budget at 8 banks: pp(2) + energy(4) + o0 + o1.
            ptag = "ep" if ab_paired else "e"
            pbufs = 2 if ab_paired else 4

            # ---------------- projections (streamed over 9 x-chunks) --------
            def emit_chunk(c):
                if c == 0:
                    xt = xt0
                elif c == 1:
                    xt = xt1
                elif c == 2:
                    xt = xt2
                else:
                    xt = sbs.tile([128, 4, BL], rdt, tag="x", bufs=3,
                                  name=f"xt{c}")
                    dma(out=xt[:], in_=x_r[:, :, c * BL:(c + 1) * BL])
                # q is only needed on extended cols [HALF, LEXT-HALF)
                qlo = max(c * BL, HALF) - c * BL
                qhi = min((c + 1) * BL, LEXT - HALF) - c * BL
                for o in range(2):
                    pq = ps.tile([128, BL], f32, tag="pp", bufs=2,
                                 name=f"pq{c}_{o}")
                    for r in range(4):
                        mm(pq[:, 0:qhi - qlo],
                           wq[:, r, o * 128:(o + 1) * 128],
                           xt[:, r, qlo:qhi], r == 0, r == 3)
                    # split the two q evacuations across scalar/vector so
                    # the pp-ring drains in parallel
                    if o == 0:
                        nc.scalar.activation(
                            q_sb[o][:, c * BL + qlo:c * BL + qhi],
                            pq[:, 0:qhi - qlo], AF.Identity,
                            bias=bq[:, o, :], scale=1.0)
                    else:
                        nc.vector.tensor_scalar_add(
                            q_sb[o][:, c * BL + qlo:c * BL + qhi],
                            pq[:, 0:qhi - qlo], bq[:, o, :])
                for o in range(2):
                    pk = ps.tile([128, BL], f32, tag="pp", bufs=2,
                                 name=f"pk{c}_{o}")
                    for r in range(4):
                        mm(pk[:], wk[:, r, o * 128:(o + 1) * 128], xt[:, r, :],
                           r == 0, r == 3)
                    nc.vector.tensor_scalar_add(
                        k_sb[o][:, c * BL:(c + 1) * BL], pk[:], bk[:, o, :])
                for lp in range(2):
                    pv = ps.tile([128, 2, C], f32, tag=ptag, bufs=pbufs,
                                 name=f"pv{c}_{lp}")
                    for j in range(2):
                        lt = lp * 2 + j
                        for r in range(4):
                            mm(pv[:, j, :], xt[:, r, lt * 128:(lt + 1) * 128],
                               wv[:, r, :], r == 0, r == 3)
                    nc.vector.tensor_copy(vts[c * 2 + lp][:], pv[:])

            emit_chunk(0)
            wo, bv, ab, ones_m = emit_late_consts()
            for c in range(1, NCH):
                emit_chunk(c)

            # ---------------- attention (software-pipelined blocks) ---------
            OPS, SPS, RBS, ORL = {}, {}, {}, {}

            def emit_attn(b):
                pts = sbs.tile([128, NKT, BL], rdt, tag="pt", bufs=2,
                               name=f"pt{b}")
                o0 = ps.tile([128, BL], f32, tag="o0", bufs=1, name=f"o0_{b}")
                o1 = ps.tile([128, BL], f32, tag="o1", bufs=1, name=f"o1_{b}")

                def qk(kt):
                    pe = ps.tile([128, BL], f32, tag="e", bufs=4,
                                 name=f"e{b}_{kt}")
                    for ct in range(2):
                        mm(pe[:],
                           k_sb[ct][:, b * BL + kt * 128:
                                    b * BL + (kt + 1) * 128],
                           q_sb[ct][:, HALF + b * BL:HALF + (b + 1) * BL],
                           ct == 0, ct == 1)
                    nc.scalar.activation(
                        pts[:, kt, :], pe[:], AF.Exp,
                        bias=ab[:, b * NKT + kt:b * NKT + kt + 1],
                        scale=1.0 / 16.0)

                def qk_pair(kp):
                    # 2-bank energy tile; one Exp ACT covers both k-tiles
                    # (the log-mask bias is per-partition and pairwise-equal
                    # for every mask setup_inputs produces - host-verified)
                    pe = ps.tile([128, 2, BL], f32, tag="ep", bufs=2,
                                 name=f"ep{b}_{kp}")
                    for j in range(2):
                        kt = 2 * kp + j
                        for ct in range(2):
                            mm(pe[:, j, :],
                               k_sb[ct][:, b * BL + kt * 128:
                                        b * BL + (kt + 1) * 128],
                               q_sb[ct][:, HALF + b * BL:HALF + (b + 1) * BL],
                               ct == 0, ct == 1)
                    nc.scalar.activation(
                        pts[:, 2 * kp:2 * kp + 2, :], pe[:], AF.Exp,
                        bias=ab[:, b * NKT + 2 * kp:b * NKT + 2 * kp + 1],
                        scale=1.0 / 16.0)

                def av(kt):
                    vtt = vts[(b * 4 + kt) // 2]
                    j = (b * 4 + kt) % 2
                    pt = pts[:, kt, :]
                    mm(o0[:], vtt[:, j, 0:128], pt, kt == 0, kt == NKT - 1)
                    mm(o1[:], vtt[:, j, 128:256], pt, kt == 0, kt == NKT - 1)

                if ab_paired:
                    qk_pair(0)
                    qk_pair(1)
                    av(0)
                    qk_pair(2)
                    av(1)
                    av(2)
                    qk_pair(3)
                    for kt in range(3, NKT):
                        av(kt)
                else:
                    for kt in range(4):
                        qk(kt)
                    for kt in range(NKT):
                        av(kt)
                        if kt + 4 < NKT:
                            qk(kt + 4)

                # pairwise bf16 tree for the row sums
                t4 = sbs.tile([128, 4, BL], rdt, tag="t4", bufs=2,
                              name=f"t4_{b}")
                nc.vector.tensor_add(t4[:], pts[:, 0:NKT:2, :],
                                     pts[:, 1:NKT:2, :])
                t2 = sbs.tile([128, 2, BL], rdt, tag="t2", bufs=2,
                              name=f"t2_{b}")
                nc.vector.tensor_add(t2[:], t4[:, 0:4:2, :], t4[:, 1:4:2, :])
                sar = sbs.tile([128, BL], rdt, tag="sar", bufs=2,
                               name=f"sar{b}")
                nc.vector.tensor_add(sar[:], t2[:, 0, :], t2[:, 1, :])
                # [128,128] ones lhsT: reduces partitions AND replicates the
                # row-sum to all 128 partitions (no gpsimd broadcast needed)
                sp = ps.tile([128, BL], f32, tag="pp", bufs=2, name=f"s{b}")
                mm(sp[:], ones_m[:], sar[:], True, True)
                OPS[b] = (o0, o1)
                SPS[b] = sp

            def emit_finA(b):
                rb = sbs.tile([128, BL], f32, tag="rbs", bufs=2, name=f"rb{b}")
                nc.vector.reciprocal_approx_fast(rb[:], SPS[b][:])
                RBS[b] = rb

            def emit_normrelu(b):
                orl = []
                for m in range(2):
                    rl = sbs.tile([128, BL], rdt, tag=f"rl{m}", bufs=2,
                                  name=f"rl{b}_{m}")
                    if bv_zero:
                        # relu(o/s + 0) == relu(o) * (1/s)   (s > 0)
                        nc.vector.scalar_tensor_tensor(
                            rl[:], OPS[b][m][:], 0.0, RBS[b][:],
                            ALU.max, ALU.mult)
                    else:
                        on = sbs.tile([128, BL], f32, tag=f"on{m}", bufs=2,
                                      name=f"on{b}_{m}")
                        nc.vector.tensor_mul(on[:], OPS[b][m][:], RBS[b][:])
                        nc.vector.tensor_scalar(
                            rl[:], on[:], bv[:, m, :], 0.0, ALU.add, ALU.max)
                    orl.append(rl)
                ORL[b] = orl

            def emit_outproj(b):
                ob = sbs.tile([128, 4, BL], rdt, tag="ob", bufs=2,
                              name=f"ob{b}")
                for v in range(4):
                    po = ps.tile([128, BL], f32, tag="pp", bufs=2,
                                 name=f"po{b}_{v}")
                    for m in range(2):
                        mm(po[:], wo[:, m, v * 128:(v + 1) * 128],
                           ORL[b][m][:], m == 0, m == 1)
                    # bo is added on the host; spread the evacuation copies
                    # across engines (gpsimd cannot read PSUM), and DMA each
                    # v-slice as it lands so the tail transfer is small
                    if v in (1, 3):
                        nc.scalar.copy(ob[:, v, :], po[:])
                    else:
                        nc.vector.tensor_copy(ob[:, v, :], po[:])
                    dma(out=out_r[:, v:v + 1, b * BL:(b + 1) * BL],
                        in_=ob[:, v:v + 1, :])

            for step in range(NBLK + 1):
                if step == NBLK:
                    # pipeline drain: keep the PE pstate hot with
                    # dependency-free dummies so the last outproj runs at
                    # full clock instead of the cold 2-3x-slower rate.
                    dps = ps.tile([128, BL], f32, tag=ptag, bufs=pbufs,
                                  name="dummy")
                    for i in range(20):
                        nc.tensor.matmul(dps[:, 0:256], wrm[:, 0:128],
                                         wrm[:, 0:256], start=True, stop=True)
                    for i in range(14):
                        nc.tensor.matmul(dps[:, 0:64], wrm[:, 0:128],
                                         wrm[:, 0:64], start=True, stop=True)
                if 1 <= step <= NBLK:
                    emit_normrelu(step - 1)
                if step < NBLK:
                    emit_attn(step)
                if step < NBLK:
                    emit_finA(step)
                if 1 <= step <= NBLK:
                    emit_outproj(step - 1)

    nc.compile()
    return nc


def get_nc(bv_zero=True, ab_paired=True):
    key = ("bf16", bv_zero, ab_paired)
    if key not in _NC_CACHE:
        _NC_CACHE[key] = _build_nc(bv_zero, ab_paired)
    return _NC_CACHE[key]


def make_core_inputs(inputs):
    """Split full inputs into 8 per-core input maps."""
    import ml_dtypes
    bf16 = ml_dtypes.bfloat16

    x1 = np.asarray(inputs["x1"], dtype=np.float32)
    mask = np.asarray(inputs["mask"], dtype=np.float32)
    wq_t = np.ascontiguousarray(
        np.asarray(inputs["Wq"], np.float32).T.astype(bf16))
    wk_t = np.ascontiguousarray(
        np.asarray(inputs["Wk"], np.float32).T.astype(bf16))
    wv_t = np.ascontiguousarray(
        np.asarray(inputs["Wv"], np.float32).T.astype(bf16))
    wo_t = np.ascontiguousarray(
        np.asarray(inputs["Wo"], np.float32).T.astype(bf16))
    bq = np.asarray(inputs["bq"], np.float32).reshape(C, 1)
    bk = np.asarray(inputs["bk"], np.float32).reshape(C, 1)
    bv = np.asarray(inputs["bv"], np.float32).reshape(C, 1)

    # padded log-mask (the reference pads mask with zeros, then adds
    # log(mask + 1e-6) to the energies)
    mp = np.pad(mask[:, 0, :], ((0, 0), (HALF, HALF)))
    lb = np.log(mp + np.float32(1e-6)).astype(np.float32)  # [B, L + 2*HALF]

    in_maps = []
    for core in range(NCORES):
        b, h = divmod(core, 2)
        s = h * LCH
        xe = np.zeros((CIN, LEXT), bf16)
        lo, hi = s - HALF, s + LCH + HALF
        slo, shi = max(lo, 0), min(hi, L)
        xe[:, slo - lo:slo - lo + (shi - slo)] = \
            x1[b, :, slo:shi].astype(bf16)
        ab = np.empty((128, NBLK * NKT), np.float32)
        for blk in range(NBLK):
            w = lb[b, s + blk * BL:s + blk * BL + WS]
            ab[:, blk * NKT:(blk + 1) * NKT] = w.reshape(NKT, 128).T
        in_maps.append({
            "x": xe, "wq_t": wq_t, "wk_t": wk_t, "wv_t": wv_t, "wo_t": wo_t,
            "bq": bq, "bk": bk, "bv": bv, "abias": ab,
        })
    return in_maps


def assemble_output(results, bo):
    out = np.empty((B, VD, L), np.float32)
    bo_col = np.asarray(bo, np.float32).reshape(VD, 1)
    for core in range(NCORES):
        b, h = divmod(core, 2)
        out[b, :, h * LCH:(h + 1) * LCH] = \
            results[core]["out"].astype(np.float32) + bo_col
    return out


LAST_RESULT = None


def kernel(**inputs):
    global LAST_RESULT
    from concourse.bass_utils import run_bass_kernel_spmd

    bv_zero = bool(np.all(np.asarray(inputs["bv"]) == 0.0))
    in_maps = make_core_inputs(inputs)
    ab0 = in_maps[0]["abias"]
    ab_paired = all(
        np.array_equal(m["abias"][:, 0::2], m["abias"][:, 1::2])
        for m in in_maps)
    nc = get_nc(bv_zero, ab_paired)
    res = run_bass_kernel_spmd(nc, in_maps, list(range(NCORES)))
    LAST_RESULT = res
    return assemble_output(res.results, inputs["bo"])
